# revision 9
# baseline (speedup 1.0000x reference)
"""Causal self-attention (B=2, T=2048, D=1024, H=16) on 8 TRN2 NeuronCores.

Sharding: 8-way tensor-parallel over heads (2 heads/core, both batches),
then one 8-core AllToAll reshards from head-channels to token-slices so each
core computes a disjoint [512, 1024] slice of the output projection.

Per-core program (SPMD, identical program, per-core data):
  core r: heads {2r, 2r+1}  -> qkv channel slice [128r : 128r+128)
          output slice      -> batch r//4, tokens [512*(r%4), 512*(r%4)+512)

bf16 matmul inputs (host-converted), fp32 PSUM accumulation, fp32 output.
Numpy-simulated end-to-end max rel err vs the fp32 reference: ~3.3e-3.
"""

import numpy as np
import ml_dtypes
from contextlib import ExitStack

import concourse.bass as bass
import concourse.tile as tile
from concourse import mybir, bacc
from concourse.bass_utils import run_bass_kernel_spmd

F32 = mybir.dt.float32
BF16 = mybir.dt.bfloat16

B, T, D, H, HD = 2, 2048, 1024, 16, 64
NC = 8  # cores
TI = B * T  # token instances = 4096
SCALE = HD ** -0.5


def build_nc() -> bass.Bass:
    nc = bacc.Bacc("TRN2", target_bir_lowering=False, debug=False, num_devices=NC)

    xf = nc.dram_tensor("xf", [TI, D], BF16, kind="ExternalInput").ap()
    wq = nc.dram_tensor("wq", [D, 128], BF16, kind="ExternalInput").ap()
    wk = nc.dram_tensor("wk", [D, 128], BF16, kind="ExternalInput").ap()
    wv = nc.dram_tensor("wv", [D, 128], BF16, kind="ExternalInput").ap()
    bq = nc.dram_tensor("bq", [128], BF16, kind="ExternalInput").ap()
    bk = nc.dram_tensor("bk", [128], BF16, kind="ExternalInput").ap()
    bv = nc.dram_tensor("bv", [128], BF16, kind="ExternalInput").ap()
    wo = nc.dram_tensor("wo", [D, D], BF16, kind="ExternalInput").ap()
    bo = nc.dram_tensor("bo", [D], BF16, kind="ExternalInput").ap()
    tri = nc.dram_tensor("tri", [128, 128], BF16, kind="ExternalInput").ap()
    eye = nc.dram_tensor("eye", [128, 128], BF16, kind="ExternalInput").ap()
    out = nc.dram_tensor("out", [512, D], F32, kind="ExternalOutput").ap()

    with tile.TileContext(nc) as tc, ExitStack() as ctx:
        const = ctx.enter_context(tc.tile_pool(name="const", bufs=1))
        qkvp = ctx.enter_context(tc.tile_pool(name="qkvp", bufs=1))
        xload = ctx.enter_context(tc.tile_pool(name="xload", bufs=5))
        xtp = ctx.enter_context(tc.tile_pool(name="xtp", bufs=2))
        vtb = ctx.enter_context(tc.tile_pool(name="vtb", bufs=2))
        ptp = ctx.enter_context(tc.tile_pool(name="ptp", bufs=4))
        rp = ctx.enter_context(tc.tile_pool(name="rp", bufs=2))
        atp = ctx.enter_context(tc.tile_pool(name="atp", bufs=3))
        aoutp = ctx.enter_context(tc.tile_pool(name="aoutp", bufs=1))
        osb = ctx.enter_context(tc.tile_pool(name="osb", bufs=2))
        psA = ctx.enter_context(tc.tile_pool(name="psA", bufs=6, space="PSUM"))
        psB = ctx.enter_context(tc.tile_pool(name="psB", bufs=2, space="PSUM"))
        dram = ctx.enter_context(tc.tile_pool(name="dram", bufs=1, space="DRAM"))

        # ---- constants / weights -------------------------------------------------
        wq_sb = const.tile([128, D], BF16)  # col 128c+m  <- wq[128c+p, m]
        wk_sb = const.tile([128, D], BF16)
        wv_sb = const.tile([128, D], BF16)
        nc.sync.dma_start(
            wq_sb[:].rearrange("p (c m) -> p c m", c=8),
            wq.rearrange("(c p) m -> p c m", p=128),
        )
        nc.sync.dma_start(
            wk_sb[:].rearrange("p (c m) -> p c m", c=8),
            wk.rearrange("(c p) m -> p c m", p=128),
        )
        nc.sync.dma_start(
            wv_sb[:].rearrange("p (c m) -> p c m", c=8),
            wv.rearrange("(c p) m -> p c m", p=128),
        )
        wo_sb = const.tile([128, 8 * D], BF16)  # col 1024c+n <- wo[128c+p, n]
        nc.sync.dma_start(
            wo_sb[:].rearrange("p (c n) -> p c n", c=8),
            wo.rearrange("(c p) n -> p c n", p=128),
        )
        bq_sb = const.tile([1, 128], BF16)
        bk_sb = const.tile([1, 128], BF16)
        bv_sb = const.tile([1, 128], BF16)
        bo_sb = const.tile([1, D], BF16)
        nc.sync.dma_start(bq_sb[:], bq[None, :])
        nc.sync.dma_start(bk_sb[:], bk[None, :])
        nc.sync.dma_start(bv_sb[:], bv[None, :])
        nc.sync.dma_start(bo_sb[:], bo[None, :])
        tri_sb = const.tile([128, 128], BF16)
        eye_sb = const.tile([128, 128], BF16)
        nc.sync.dma_start(tri_sb[:], tri[:])
        nc.sync.dma_start(eye_sb[:], eye[:])
        ones_sb = const.tile([1, 512], BF16)
        nc.vector.memset(ones_sb[:], 1.0)

        # Q^T / K^T, channels(128) x token-instances(4096)
        qt_sb = qkvp.tile([128, TI], BF16)
        kt_sb = qkvp.tile([128, TI], BF16)
        # V' : [kpos(128), 32 ktiles x (2 heads x 65)]; col 130*kt + 65*h + d,
        # d==64 is the ones column (softmax denominator trick)
        vp_sb = qkvp.tile([128, 32 * 130], BF16)
        vp_ones = vp_sb.rearrange("p (kt h d) -> p kt h d", kt=32, h=2, d=65)[
            :, :, :, 64:65
        ]
        nc.vector.memset(vp_ones, 1.0)

        a2a_in = dram.tile([1024, 512], BF16)
        a2a_out = dram.tile([1024, 512], BF16)

        # ---- phase A/B: x^T then QKV projections, per 512-token block ------------
        for b in range(B):
            for blk in range(4):
                base = 2048 * b + 512 * blk
                xts = []
                for i in range(4):
                    x_t = xload.tile([128, D], BF16, name="x_t")
                    nc.sync.dma_start(x_t[:], xf[base + 128 * i : base + 128 * (i + 1), :])
                    xts.append(x_t)
                xT = xtp.tile([128, 8 * 512], BF16)  # col 512c + t
                for c in range(8):
                    ps = psA.tile([128, 512], BF16, name="ps_t", tag="ps")
                    for i in range(4):
                        nc.tensor.transpose(
                            ps[:, 128 * i : 128 * (i + 1)],
                            xts[i][:, 128 * c : 128 * (c + 1)],
                            eye_sb[:],
                        )
                    eng = nc.vector if c % 2 == 0 else nc.scalar
                    if eng is nc.vector:
                        eng.tensor_copy(xT[:, 512 * c : 512 * (c + 1)], ps[:])
                    else:
                        eng.copy(xT[:, 512 * c : 512 * (c + 1)], ps[:])

                # projections: psum[128 ch, 512 tok] accumulated over 8 e-chunks
                for w_sb, b_sb, which in (
                    (wq_sb, bq_sb, "q"),
                    (wk_sb, bk_sb, "k"),
                    (wv_sb, bv_sb, "v"),
                ):
                    ps = psA.tile([128, 512], F32, name="ps_p", tag="ps")
                    for c in range(8):
                        nc.tensor.matmul(
                            ps[:],
                            w_sb[:, 128 * c : 128 * (c + 1)],
                            xT[:, 512 * c : 512 * (c + 1)],
                            start=(c == 0),
                            stop=False,
                        )
                    nc.tensor.matmul(
                        ps[:], b_sb[:], ones_sb[:], start=False, stop=True
                    )
                    if which == "q":
                        nc.scalar.copy(qt_sb[:, base : base + 512], ps[:])
                    elif which == "k":
                        nc.scalar.copy(kt_sb[:, base : base + 512], ps[:])
                    else:
                        vt_blk = vtb.tile([128, 512], BF16)
                        nc.scalar.copy(vt_blk[:], ps[:])
                        # V'[tok, ch] tiles via PE transpose
                        ps2 = psA.tile([128, 512], BF16, name="ps_vt", tag="ps")
                        for i in range(4):
                            nc.tensor.transpose(
                                ps2[:, 128 * i : 128 * (i + 1)],
                                vt_blk[:, 128 * i : 128 * (i + 1)],
                                eye_sb[:],
                            )
                        kt0 = 16 * b + 4 * blk
                        dst = vp_sb[:, 130 * kt0 : 130 * (kt0 + 4)].rearrange(
                            "p (kt h d) -> p kt h d", kt=4, h=2, d=65
                        )[:, :, :, :64]
                        src = ps2.rearrange("p (i h d) -> p i h d", i=4, h=2, d=64)
                        nc.vector.tensor_copy(dst, src)

        # ---- phase C: attention, per (batch, head, 512-query-block) --------------
        for b in range(B):
            for h in range(2):
                hr = 64 * h
                for j in range(4):
                    qbase = 2048 * b + 512 * j
                    av = psB.tile([128, 512], F32, name="av")
                    nkb = 4 * j + 4
                    for kb in range(nkb):
                        m = kb - 4 * j  # >=0 on diagonal super-block
                        off = 128 * m if m >= 0 else 0
                        ps_s = psA.tile([128, 512], F32, name="ps_s", tag="ps")
                        nc.tensor.matmul(
                            ps_s[:, off:],
                            kt_sb[hr : hr + 64, 2048 * b + 128 * kb : 2048 * b + 128 * (kb + 1)],
                            qt_sb[hr : hr + 64, qbase + off : qbase + 512],
                            start=True,
                            stop=True,
                        )
                        pt = ptp.tile([128, 512], BF16, name="pt")
                        nc.scalar.activation(
                            pt[:, off:],
                            ps_s[:, off:],
                            mybir.ActivationFunctionType.Exp,
                            scale=SCALE,
                        )
                        if m >= 0:
                            nc.vector.tensor_mul(
                                pt[:, off : off + 128],
                                pt[:, off : off + 128],
                                tri_sb[:],
                            )
                        nc.tensor.matmul(
                            av[0:65, off:],
                            vp_sb[:, 130 * (16 * b + kb) + 65 * h : 130 * (16 * b + kb) + 65 * h + 65],
                            pt[:, off:],
                            start=(kb == 0),
                            stop=(kb == nkb - 1),
                        )
                    recip = rp.tile([1, 512], F32, name="recip")
                    nc.vector.reciprocal(recip[:], av[64:65, :])
                    rbc = rp.tile([64, 512], F32, name="rbc")
                    nc.gpsimd.partition_broadcast(rbc[:], recip[:])
                    at = atp.tile([64, 512], BF16, name="at")
                    nc.vector.tensor_mul(at[:], av[0:64, :], rbc[:])
                    s = 4 * b + j  # destination core index (token-slice owner)
                    nc.sync.dma_start(
                        a2a_in[128 * s + 64 * h : 128 * s + 64 * h + 64, :], at[:]
                    )

        # ---- phase D: reshard heads->tokens --------------------------------------
        nc.gpsimd.collective_compute(
            "AllToAll",
            mybir.AluOpType.bypass,
            replica_groups=[list(range(NC))],
            ins=[a2a_in.opt()],
            outs=[a2a_out.opt()],
        )

        # ---- phase E: output projection for my 512-token slice -------------------
        attn2 = aoutp.tile([128, 8 * 512], BF16)  # col 512c + t  (= attn^T chunks)
        for c in range(8):
            nc.sync.dma_start(
                attn2[:, 512 * c : 512 * (c + 1)],
                a2a_out[128 * c : 128 * (c + 1), :],
            )
        for mt in range(4):
            o_t = osb.tile([128, D], F32, name="o_t")
            for nh in range(2):
                ps_o = psA.tile([128, 512], F32, name="ps_o", tag="ps")
                for c in range(8):
                    nc.tensor.matmul(
                        ps_o[:],
                        attn2[:, 512 * c + 128 * mt : 512 * c + 128 * (mt + 1)],
                        wo_sb[:, 1024 * c + 512 * nh : 1024 * c + 512 * (nh + 1)],
                        start=(c == 0),
                        stop=False,
                    )
                nc.tensor.matmul(
                    ps_o[:],
                    ones_sb[:, 0:128],
                    bo_sb[:, 512 * nh : 512 * (nh + 1)],
                    start=False,
                    stop=True,
                )
                nc.scalar.copy(o_t[:, 512 * nh : 512 * (nh + 1)], ps_o[:])
            nc.sync.dma_start(out[128 * mt : 128 * (mt + 1), :], o_t[:])

    nc.compile()
    return nc


_NC_CACHE = None


def _get_nc():
    global _NC_CACHE
    if _NC_CACHE is None:
        _NC_CACHE = build_nc()
    return _NC_CACHE


def _b16(a):
    return np.ascontiguousarray(np.asarray(a, np.float32).astype(ml_dtypes.bfloat16))


def make_in_maps(x, Wq, bq, Wk, bk, Wv, bv, Wo, bo):
    xf = _b16(np.asarray(x, np.float32).reshape(TI, D))
    Wq, Wk, Wv, Wo = _b16(Wq), _b16(Wk), _b16(Wv), _b16(Wo)
    bq, bk, bv, bo = _b16(bq), _b16(bk), _b16(bv), _b16(bo)
    tri = np.triu(np.ones((128, 128), ml_dtypes.bfloat16))  # tri[ki, qi] = ki <= qi
    eye = np.eye(128, dtype=ml_dtypes.bfloat16)
    in_maps = []
    for r in range(NC):
        ch = slice(128 * r, 128 * (r + 1))
        in_maps.append(
            {
                "xf": xf,
                "wq": np.ascontiguousarray(Wq[:, ch]),
                "wk": np.ascontiguousarray(Wk[:, ch]),
                "wv": np.ascontiguousarray(Wv[:, ch]),
                "bq": np.ascontiguousarray(bq[ch]),
                "bk": np.ascontiguousarray(bk[ch]),
                "bv": np.ascontiguousarray(bv[ch]),
                "wo": Wo,
                "bo": bo,
                "tri": tri,
                "eye": eye,
            }
        )
    return in_maps


def assemble(results):
    out = np.empty((B, T, D), np.float32)
    for r in range(NC):
        out[r // 4, 512 * (r % 4) : 512 * (r % 4 + 1), :] = results[r]["out"]
    return out


def run(inputs, trace=False, **kw):
    nc = _get_nc()
    in_maps = make_in_maps(**inputs)
    res = run_bass_kernel_spmd(nc, in_maps, core_ids=list(range(NC)), trace=trace, **kw)
    return assemble(res.results), res


def kernel(**inputs) -> np.ndarray:
    out, _ = run(inputs)
    return out
